# revision 10
# baseline (speedup 1.0000x reference)
"""GAT layer (gnn_message_passing) Trainium2 kernel, 8-core edge-parallel.

Strategy (dst-sorted edge-centric):
  - Host: fold attention weight vectors into small matrices; bin-pack nodes
    into 392 blocks of 128 (LPT on in-degree) so every block has a near-equal
    number of incoming edges; sort edges by destination block; within a block
    split edges by gathered-row index (<32768 vs >=32768, the int16 limit of
    the dma_gather custom instruction) into two contiguous sub-lists.
  - Device (SPMD, 8 cores, 49 blocks each):
      Phase 1: x = node_features @ W projected once per core into a bf16
               DRAM table xt [50176, 128] (256B rows).
      Phase 2 per block: dma_gather the block's edge source rows, build
               one-hot destination masks with is_equal, compute attention
               logits (edge term via mul+reduce, dst term via PE transpose +
               matmul), softmax without max-subtraction (logits are tiny),
               and accumulate messages + softmax stats with a single fused
               PE matmul per 128-edge chunk. Self loops handled analytically.
  - Host: un-permute rows of the gathered per-core outputs.
"""

import numpy as np

try:
    import concourse.bacc as bacc_mod  # noqa: F401
except Exception:  # pragma: no cover
    import sys

    for p in ("/opt/trn_rl_repo", "/root/.axon_site/_ro/trn_rl_repo"):
        if p not in sys.path:
            sys.path.insert(0, p)
    import concourse.bacc as bacc_mod

import ml_dtypes
import concourse.bass as bass
import concourse.tile as tile
from concourse import mybir
from concourse.bass_utils import run_bass_kernel_spmd

F32 = mybir.dt.float32
BF16 = mybir.dt.bfloat16
I16 = mybir.dt.int16
BF = ml_dtypes.bfloat16

CORES = 8
NEG_ATT = 0.2
NEG_OUT = 0.01


class Cfg:
    def __init__(self, N, E, SPLIT, NBLK, F=128, H=4, C=32, ED=16):
        assert NBLK % CORES == 0 and SPLIT % 128 == 0
        self.N, self.E, self.SPLIT, self.NBLK = N, E, SPLIT, NBLK
        self.F, self.H, self.C, self.ED = F, H, C, ED
        self.HC = H * C
        self.NPAD = NBLK * 128
        self.ABLK = SPLIT // 128
        self.BBLK = NBLK - self.ABLK
        assert self.ABLK % CORES == 0 and self.BBLK % CORES == 0
        self.ASLOTS = self.ABLK // CORES
        self.BSLOTS = self.BBLK // CORES
        self.BPC = self.ASLOTS + self.BSLOTS
        # nA/nB (chunks per block for the two gather halves) set in prep()
        self.nA = None
        self.nB = None

    @property
    def G(self):
        return self.nA + self.nB


def full_cfg():
    # NBLK=400 -> NPAD=51200 >= 50000, ABLK=256, BBLK=144 (18/core), BPC=50.
    return Cfg(N=50000, E=800000, SPLIT=32768, NBLK=400)


def _wrap_idx(vals):
    """int16 index list -> [128, len/16] wrapped layout for dma_gather."""
    m = vals.shape[-1]
    assert m % 16 == 0
    base = vals.reshape(*vals.shape[:-1], m // 16, 16)
    base = np.swapaxes(base, -1, -2)  # [..., 16, m/16]
    return np.concatenate([base] * 8, axis=-2).astype(np.int16)  # [...,128,m/16]


def prep(cfg, node_features, edge_index, edge_attr, W, W_edge,
         att_src, att_dst, att_edge, bias):
    N, E, H, C, ED, F = cfg.N, cfg.E, cfg.H, cfg.C, cfg.ED, cfg.F
    NPAD, NBLK, SPLIT = cfg.NPAD, cfg.NBLK, cfg.SPLIT

    src = np.asarray(edge_index[0], np.int64)
    dst = np.asarray(edge_index[1], np.int64)
    ea = np.asarray(edge_attr, np.float32)

    # ---- fold weights ----
    V = np.einsum("dhc,hc->dh", np.asarray(W_edge, np.float32).reshape(ED, H, C),
                  np.asarray(att_edge, np.float32))  # [ED, H]

    # ---- LPT bin-pack nodes into blocks by in-degree ----
    deg = np.bincount(dst, minlength=N)
    order = np.argsort(-deg, kind="stable")
    import heapq
    heap = [(0, b) for b in range(NBLK)]
    heapq.heapify(heap)
    counts = np.zeros(NBLK, np.int64)
    blk_of = np.empty(N, np.int64)
    for n in order:
        load, b = heapq.heappop(heap)
        blk_of[n] = b
        counts[b] += 1
        if counts[b] < 128:
            heapq.heappush(heap, (int(load) + int(deg[n]), b))
    # position within block
    prow = np.empty(N, np.int64)
    pos_ctr = np.zeros(NBLK, np.int64)
    for n in np.argsort(blk_of, kind="stable"):
        b = blk_of[n]
        prow[n] = b * 128 + pos_ctr[b]
        pos_ctr[b] += 1

    srow = prow[src]
    drow = prow[dst]
    eblk = drow // 128
    half = (srow >= SPLIT).astype(np.int64)

    key = eblk * 2 + half
    eorder = np.argsort(key, kind="stable")
    ks = key[eorder]
    counts2 = np.bincount(ks, minlength=NBLK * 2)
    starts = np.zeros(NBLK * 2, np.int64)
    starts[1:] = np.cumsum(counts2)[:-1]
    rank = np.arange(E) - starts[ks]

    LA = counts2[0::2]
    LB = counts2[1::2]
    nA = max(1, int(np.ceil(LA.max() / 128)))
    nB = max(1, int(np.ceil(LB.max() / 128)))
    cfg.nA, cfg.nB = nA, nB
    G = cfg.G

    # ---- grids ----
    gidxA = np.zeros((NBLK, nA * 128), np.int64)
    gidxB = np.zeros((NBLK, nB * 128), np.int64)
    gdrt = np.full((NBLK, 128, G), 128.0, np.float32)
    geao = np.zeros((NBLK, 128, G, ED + 1), np.float32)

    es, eh, er = eblk[eorder], half[eorder], rank
    esrow, edrow = srow[eorder], drow[eorder]
    eea = ea[eorder]

    a_m = eh == 0
    gidxA[es[a_m], er[a_m]] = esrow[a_m]
    b_m = ~a_m
    gidxB[es[b_m], er[b_m]] = esrow[b_m] - SPLIT

    gchunk = np.where(a_m, er // 128, nA + er // 128)
    gj = er % 128
    gdrt[es, gj, gchunk] = (edrow % 128).astype(np.float32)
    geao[es, gj, gchunk, :ED] = eea
    geao[es, gj, gchunk, ED] = 1.0

    gidxS = np.empty((NBLK, 128), np.int64)
    gidxS[:] = np.arange(NBLK)[:, None] * 128 + np.arange(128)[None, :]
    gidxS[cfg.ABLK:] -= SPLIT

    # ---- per-core slicing: core c owns A-blocks [c*AS,(c+1)*AS) + B ----
    AS, BS = cfg.ASLOTS, cfg.BSLOTS
    core_blocks = []
    for c in range(CORES):
        abl = list(range(c * AS, (c + 1) * AS))
        bbl = list(range(cfg.ABLK + c * BS, cfg.ABLK + (c + 1) * BS))
        core_blocks.append(abl + bbl)
    core_blocks = np.array(core_blocks)  # [CORES, BPC]

    # ---- node feature table (permuted, transposed, bf16) ----
    nfT = np.zeros((F, NPAD), np.float32)
    nf = np.asarray(node_features, np.float32)
    inv = np.full(NPAD, -1, np.int64)
    inv[prow] = np.arange(N)
    valid = inv >= 0
    nfT[:, valid] = nf[inv[valid]].T

    att_src_rep = np.tile(np.asarray(att_src, np.float32)[None], (128, 1, 1))
    att_dst_rep = np.tile(np.asarray(att_dst, np.float32)[None], (128, 1, 1))
    V_rep = np.tile(V.T[None], (128, 1, 1))  # [128, H, ED]
    bias_rep = np.tile(np.asarray(bias, np.float32)[None], (128, 1))
    iota_row = np.tile(np.arange(128, dtype=np.float32)[None], (128, 1))
    ident = np.eye(128, dtype=np.float32)

    in_maps = []
    for c in range(CORES):
        bl = core_blocks[c]
        in_maps.append({
            "nfT": nfT.astype(BF),
            "Wb": np.asarray(W, np.float32).astype(BF),
            "idxA": _wrap_idx(gidxA[bl]),
            "idxB": _wrap_idx(gidxB[bl]),
            "idxS": _wrap_idx(gidxS[bl]),
            "dstrelT": gdrt[bl].astype(BF),
            "eaones": geao[bl].astype(BF),
            "att_src_rep": att_src_rep.astype(BF),
            "att_dst_rep": att_dst_rep.astype(BF),
            "V_rep": V_rep.astype(BF),
            "bias_rep": bias_rep,
            "iota_row": iota_row.astype(BF),
            "ident": ident.astype(BF),
        })

    meta = dict(prow=prow, core_blocks=core_blocks, valid=valid, inv=inv)
    return in_maps, meta


def vw(ap, pairs, extra_offset=0):
    """Manual AP view: keep tensor, adjust offset, replace ap pairs."""
    return bass.AP(tensor=ap.tensor, offset=ap.offset + extra_offset, ap=pairs)


def build(cfg):
    NPAD, NBLK, SPLIT = cfg.NPAD, cfg.NBLK, cfg.SPLIT
    BPC, AS, G, nA, nB = cfg.BPC, cfg.ASLOTS, cfg.G, cfg.nA, cfg.nB
    F, H, C, ED = cfg.F, cfg.H, cfg.C, cfg.ED
    HC = cfg.HC
    EW = ED + 1            # ea + ones
    RW = HC + H + EW       # rhs width: wmsg | ex | ea,ones = 128+4+17
    NCHUNK_X = NPAD // 128

    nc = bacc_mod.Bacc(num_swdge_queues=4)

    nfT = nc.dram_tensor("nfT", [F, NPAD], BF16, kind="ExternalInput")
    Wb = nc.dram_tensor("Wb", [F, HC], BF16, kind="ExternalInput")
    idxA = nc.dram_tensor("idxA", [BPC, 128, nA * 8], I16, kind="ExternalInput")
    idxB = nc.dram_tensor("idxB", [BPC, 128, nB * 8], I16, kind="ExternalInput")
    idxS = nc.dram_tensor("idxS", [BPC, 128, 8], I16, kind="ExternalInput")
    dstrelT = nc.dram_tensor("dstrelT", [BPC, 128, G], BF16, kind="ExternalInput")
    eaones = nc.dram_tensor("eaones", [BPC, 128, G, EW], BF16, kind="ExternalInput")
    att_src_rep = nc.dram_tensor("att_src_rep", [128, H, C], BF16, kind="ExternalInput")
    att_dst_rep = nc.dram_tensor("att_dst_rep", [128, H, C], BF16, kind="ExternalInput")
    V_rep = nc.dram_tensor("V_rep", [128, H, ED], BF16, kind="ExternalInput")
    bias_rep = nc.dram_tensor("bias_rep", [128, HC], F32, kind="ExternalInput")
    iota_row = nc.dram_tensor("iota_row", [128, 128], BF16, kind="ExternalInput")
    ident = nc.dram_tensor("ident", [128, 128], BF16, kind="ExternalInput")
    out = nc.dram_tensor("out", [BPC, 128, HC], F32, kind="ExternalOutput")

    with tile.TileContext(nc) as tc:
        with (
            tc.tile_pool(name="dram", bufs=1, space="DRAM") as dpool,
            tc.tile_pool(name="const", bufs=1) as cpool,
            tc.tile_pool(name="p1", bufs=4) as p1pool,
            tc.tile_pool(name="p1ps", bufs=2, space="PSUM") as p1ps,
            tc.tile_pool(name="big", bufs=2) as bpool,
            tc.tile_pool(name="med", bufs=2) as mpool,
            tc.tile_pool(name="sml", bufs=3) as spool,
            tc.tile_pool(name="msk", bufs=4) as kpool,
            tc.tile_pool(name="psA", bufs=2, space="PSUM") as psA,
            tc.tile_pool(name="psB", bufs=1, space="PSUM") as psB,
            tc.tile_pool(name="psT", bufs=2, space="PSUM") as psT,
        ):
            xt = dpool.tile([NPAD, HC], BF16)

            # constants
            c_w = cpool.tile([F, HC], BF16)
            nc.sync.dma_start(out=c_w[:], in_=Wb[:])
            c_asrc = cpool.tile([128, H, C], BF16)
            nc.sync.dma_start(out=c_asrc[:], in_=att_src_rep[:])
            c_adst = cpool.tile([128, H, C], BF16)
            nc.sync.dma_start(out=c_adst[:], in_=att_dst_rep[:])
            c_v = cpool.tile([128, H, ED], BF16)
            nc.sync.dma_start(out=c_v[:], in_=V_rep[:])
            c_bias = cpool.tile([128, HC], F32)
            nc.sync.dma_start(out=c_bias[:], in_=bias_rep[:])
            c_iota = cpool.tile([128, 128], BF16)
            nc.sync.dma_start(out=c_iota[:], in_=iota_row[:])
            c_id = cpool.tile([128, 128], BF16)
            nc.sync.dma_start(out=c_id[:], in_=ident[:])

            # ---- Phase 1: xt = (nf @ W) in bf16 ----
            for k in range(NCHUNK_X):
                lt = p1pool.tile([F, 128], BF16, tag="p1lhs")
                nc.sync.dma_start(out=lt[:], in_=nfT[:, k * 128:(k + 1) * 128])
                ps = p1ps.tile([128, HC], F32, tag="p1ps")
                nc.tensor.matmul(out=ps[:], lhsT=lt[:], rhs=c_w[:],
                                 start=True, stop=True)
                xb = p1pool.tile([128, HC], BF16, tag="p1out")
                if k % 2 == 0:
                    nc.vector.tensor_copy(out=xb[:], in_=ps[:])
                else:
                    nc.scalar.copy(out=xb[:], in_=ps[:])
                nc.sync.dma_start(out=xt[k * 128:(k + 1) * 128, :], in_=xb[:])

            # ---- Phase 2: per block ----
            for b in range(BPC):
                # --- input DMAs ---
                ia = spool.tile([128, nA * 8], I16, tag="ia")
                nc.sync.dma_start(out=ia[:], in_=idxA[b])
                ib = spool.tile([128, nB * 8], I16, tag="ib")
                nc.sync.dma_start(out=ib[:], in_=idxB[b])
                isf = spool.tile([128, 8], I16, tag="isf")
                nc.sync.dma_start(out=isf[:], in_=idxS[b])
                drt = spool.tile([128, G], BF16, tag="drt")
                nc.sync.dma_start(out=drt[:], in_=dstrelT[b])

                rhs = bpool.tile([128, G, RW], BF16, tag="rhs")
                nc.sync.dma_start(
                    out=rhs[:, :, HC + H:RW],
                    in_=eaones[b],
                )

                # --- gathers (SWDGE Q7) ---
                xg = bpool.tile([128, G, HC], BF16, tag="xg")
                nc.gpsimd.dma_gather(
                    xg[:, 0:nA, :], xt[:], ia[:], nA * 128, nA * 128, HC,
                    queue_num=(2 * b) % 4, single_packet=False)
                nc.gpsimd.dma_gather(
                    xg[:, nA:G, :], xt[SPLIT:, :], ib[:], nB * 128, nB * 128, HC,
                    queue_num=(2 * b + 1) % 4, single_packet=False)
                xs = mpool.tile([128, 1, HC], BF16, tag="xs")
                xt_self = xt[:] if b < AS else xt[SPLIT:, :]
                nc.gpsimd.dma_gather(
                    xs[:], xt_self, isf[:], 128, 128, HC,
                    queue_num=(2 * b) % 4, single_packet=False)

                # --- maskT [j, G, i] = (dst_rel[j,g] == i) ---
                maskT = bpool.tile([128, G, 128], BF16, tag="maskT")
                drt_b = vw(drt[:], [drt[:].ap[0], [1, G], [0, 128]])
                iota_b = vw(c_iota[:], [c_iota[:].ap[0], [0, G], [1, 128]])
                nc.vector.tensor_tensor(out=maskT[:], in0=drt_b, in1=iota_b,
                                        op=mybir.AluOpType.is_equal)

                # --- a_src_e [j, G, H] = sum_c xg*att_src ---
                t_as = bpool.tile([128, G, H, C], BF16, tag="t_as")
                xg4 = vw(xg[:], [xg[:].ap[0], [HC, G], [C, H], [1, C]])
                asrc_b = vw(c_asrc[:], [c_asrc[:].ap[0], [0, G], [C, H], [1, C]])
                nc.vector.tensor_tensor(out=t_as[:], in0=xg4, in1=asrc_b,
                                        op=mybir.AluOpType.mult)
                a_src_e = mpool.tile([128, G, H], F32, tag="a_src_e")
                nc.vector.reduce_sum(out=a_src_e[:], in_=t_as[:],
                                     axis=mybir.AxisListType.X)

                # --- a_e [j, G, H] = sum_d ea*V ---
                t_ae = mpool.tile([128, G, H, ED], BF16, tag="t_ae")
                ea_b = vw(rhs[:], [rhs[:].ap[0], [RW, G], [0, H], [1, ED]],
                          extra_offset=HC + H)
                v_b = vw(c_v[:], [c_v[:].ap[0], [0, G], [ED, H], [1, ED]])
                nc.vector.tensor_tensor(out=t_ae[:], in0=ea_b, in1=v_b,
                                        op=mybir.AluOpType.mult)
                a_e = mpool.tile([128, G, H], F32, tag="a_e")
                nc.vector.reduce_sum(out=a_e[:], in_=t_ae[:],
                                     axis=mybir.AxisListType.X)

                # --- per-node (block) a_src/a_dst from self rows ---
                t_bs = mpool.tile([128, 2, H, C], BF16, tag="t_bs")
                xs4 = vw(xs[:], [xs[:].ap[0], [0, 2], [C, H], [1, C]])
                ad2 = vw(c_adst[:], [c_adst[:].ap[0], [0, 1], [C, H], [1, C]])
                as2 = vw(c_asrc[:], [c_asrc[:].ap[0], [0, 1], [C, H], [1, C]])
                # stack att_dst & att_src comparisons: do separately (2 ops)
                nc.vector.tensor_tensor(
                    out=t_bs[:, 0:1], in0=vw(xs[:], [xs[:].ap[0], [0, 1], [C, H], [1, C]]),
                    in1=ad2, op=mybir.AluOpType.mult)
                nc.vector.tensor_tensor(
                    out=t_bs[:, 1:2], in0=vw(xs[:], [xs[:].ap[0], [0, 1], [C, H], [1, C]]),
                    in1=as2, op=mybir.AluOpType.mult)
                blkv = spool.tile([128, 2, H], F32, tag="blkv")  # [dst, src]
                nc.vector.reduce_sum(out=blkv[:], in_=t_bs[:],
                                     axis=mybir.AxisListType.X)
                a_dst_bf = spool.tile([128, H], BF16, tag="a_dst_bf")
                nc.vector.tensor_copy(out=a_dst_bf[:], in_=blkv[:, 0, :])

                # --- per chunk: mask transpose + a_dst expansion ---
                ps3 = psB.tile([128, G, H], F32, tag="ps3")
                for g in range(G):
                    pst = psT.tile([128, 128], BF16, tag="pst")
                    nc.tensor.transpose(out=pst[:], in_=maskT[:, g, :],
                                        identity=c_id[:])
                    msb = kpool.tile([128, 128], BF16, tag="msb")
                    nc.scalar.copy(out=msb[:], in_=pst[:])
                    nc.tensor.matmul(out=ps3[:, g, :], lhsT=msb[:],
                                     rhs=a_dst_bf[:], start=True, stop=True)

                # --- alpha / ex ---
                alpha = mpool.tile([128, G, H], F32, tag="alpha")
                nc.vector.tensor_add(out=alpha[:], in0=ps3[:], in1=a_src_e[:])
                nc.vector.tensor_add(out=alpha[:], in0=alpha[:], in1=a_e[:])
                lrel = mpool.tile([128, G, H], F32, tag="lrel")
                nc.vector.tensor_scalar_mul(out=lrel[:], in0=alpha[:],
                                            scalar1=NEG_ATT)
                nc.vector.tensor_tensor(out=lrel[:], in0=lrel[:], in1=alpha[:],
                                        op=mybir.AluOpType.max)
                nc.scalar.activation(out=rhs[:, :, HC:HC + H], in_=lrel[:],
                                     func=mybir.ActivationFunctionType.Exp)

                # --- wmsg = xg * ex ---
                ex_b = vw(rhs[:], [rhs[:].ap[0], [RW, G], [1, H], [0, C]],
                          extra_offset=HC)
                wout = vw(rhs[:], [rhs[:].ap[0], [RW, G], [C, H], [1, C]])
                nc.vector.tensor_tensor(out=wout, in0=xg4, in1=ex_b,
                                        op=mybir.AluOpType.mult)

                # --- main accumulation matmul ---
                psm = psA.tile([128, RW], F32, tag="psm")
                for g in range(G):
                    nc.tensor.matmul(out=psm[:], lhsT=maskT[:, g, :],
                                     rhs=rhs[:, g, :],
                                     start=(g == 0), stop=(g == G - 1))

                # --- self loop + normalize ---
                cntc = spool.tile([128, 1], F32, tag="cntc")
                nc.vector.tensor_scalar_max(out=cntc[:], in0=psm[:, RW - 1:RW],
                                            scalar1=1.0)
                rcnt = spool.tile([128, 1], F32, tag="rcnt")
                nc.vector.reciprocal(out=rcnt[:], in_=cntc[:])
                lattr = spool.tile([128, ED], F32, tag="lattr")
                nc.vector.tensor_scalar_mul(out=lattr[:],
                                            in0=psm[:, HC + H:HC + H + ED],
                                            scalar1=rcnt[:, 0:1])
                t_al = spool.tile([128, H, ED], F32, tag="t_al")
                lattr_b = vw(lattr[:], [lattr[:].ap[0], [0, H], [1, ED]])
                v_b2 = vw(c_v[:], [c_v[:].ap[0], [ED, H], [1, ED]])
                nc.vector.tensor_tensor(out=t_al[:], in0=lattr_b, in1=v_b2,
                                        op=mybir.AluOpType.mult)
                a_el = spool.tile([128, H], F32, tag="a_el")
                nc.vector.reduce_sum(out=a_el[:], in_=t_al[:],
                                     axis=mybir.AxisListType.X)
                alf = spool.tile([128, H], F32, tag="alf")
                nc.vector.tensor_add(out=alf[:], in0=blkv[:, 0, :], in1=blkv[:, 1, :])
                nc.vector.tensor_add(out=alf[:], in0=alf[:], in1=a_el[:])
                alf2 = spool.tile([128, H], F32, tag="alf2")
                nc.vector.tensor_scalar_mul(out=alf2[:], in0=alf[:], scalar1=NEG_ATT)
                nc.vector.tensor_tensor(out=alf2[:], in0=alf2[:], in1=alf[:],
                                        op=mybir.AluOpType.max)
                exl = spool.tile([128, H], F32, tag="exl")
                nc.scalar.activation(out=exl[:], in_=alf2[:],
                                     func=mybir.ActivationFunctionType.Exp)

                den = spool.tile([128, H], F32, tag="den")
                nc.vector.tensor_add(out=den[:], in0=psm[:, HC:HC + H], in1=exl[:])
                rden = spool.tile([128, H], F32, tag="rden")
                nc.vector.reciprocal(out=rden[:], in_=den[:])

                smsg = mpool.tile([128, HC], F32, tag="smsg")
                exl_b = vw(exl[:], [exl[:].ap[0], [1, H], [0, C]])
                xs2 = vw(xs[:], [xs[:].ap[0], [C, H], [1, C]])
                nc.vector.tensor_tensor(out=smsg[:], in0=xs2, in1=exl_b,
                                        op=mybir.AluOpType.mult)
                agg = mpool.tile([128, HC], F32, tag="agg")
                nc.vector.tensor_add(out=agg[:], in0=psm[:, 0:HC], in1=smsg[:])
                rden_b = vw(rden[:], [rden[:].ap[0], [1, H], [0, C]])
                nc.vector.tensor_tensor(out=agg[:], in0=agg[:], in1=rden_b,
                                        op=mybir.AluOpType.mult)
                nc.vector.tensor_add(out=agg[:], in0=agg[:], in1=c_bias[:])
                osb = mpool.tile([128, HC], F32, tag="osb")
                nc.scalar.mul(out=osb[:], in_=agg[:], mul=NEG_OUT)
                nc.vector.tensor_tensor(out=osb[:], in0=osb[:], in1=agg[:],
                                        op=mybir.AluOpType.max)
                nc.sync.dma_start(out=out[b], in_=osb[:])

    nc.finalize()
    return nc


def assemble(cfg, meta, results):
    """Gather per-core outputs back to full [N, HC] float32."""
    NPAD = cfg.NPAD
    flat = np.zeros((NPAD, cfg.HC), np.float32)
    for c in range(CORES):
        o = results[c]["out"]  # [BPC, 128, HC]
        bl = meta["core_blocks"][c]
        for s, b in enumerate(bl):
            flat[b * 128:(b + 1) * 128] = o[s]
    y = np.empty((cfg.N, cfg.HC), np.float32)
    y[meta["inv"][meta["valid"]]] = flat[meta["valid"]]
    return y


_BUILD_CACHE = {}


def kernel(**inputs):
    cfg = full_cfg()
    in_maps, meta = prep(cfg, **inputs)
    ckey = (cfg.N, cfg.E, cfg.nA, cfg.nB)
    if ckey in _BUILD_CACHE:
        nc = _BUILD_CACHE[ckey]
    else:
        nc = build(cfg)
        _BUILD_CACHE[ckey] = nc
    res = run_bass_kernel_spmd(nc, in_maps, core_ids=list(range(CORES)))
    return assemble(cfg, meta, res.results)


# revision 13
# speedup vs baseline: 1.3252x; 1.3252x over previous
"""GAT layer (gnn_message_passing) Trainium2 kernel, 8-core edge-parallel.

Strategy (dst-sorted edge-centric):
  - Host: fold attention weight vectors into small matrices; bin-pack nodes
    into NBLK blocks of 128 (LPT on in-degree) so every block has a near-equal
    number of incoming edges; sort edges by destination block; within a block
    split edges by gathered-row index (<32768 vs >=32768, the int16 limit of
    the dma_gather custom instruction) into two contiguous sub-lists.
  - Device (SPMD, 8 cores, NBLK/8 blocks each):
      Phase 1: x = node_features @ W projected once per core into a bf16
               DRAM table xt (256B rows).
      Phase 2: consolidated dma_gathers fetch edge source rows for groups of
               K blocks at a time; per block, one-hot destination masks are
               built with is_equal; the dst attention term is expanded per
               edge via a K=1 broadcast matmul + is_equal + small matmul;
               softmax runs without max-subtraction (logits are tiny); one
               fused PE matmul per 128-edge chunk accumulates messages,
               softmax denominators, self-loop edge_attr sums and counts.
               Self loops are handled analytically per block.
  - Host: un-permute rows of the gathered per-core outputs.
"""

import numpy as np

try:
    import concourse.bacc as bacc_mod  # noqa: F401
except Exception:  # pragma: no cover
    import sys

    for p in ("/opt/trn_rl_repo", "/root/.axon_site/_ro/trn_rl_repo"):
        if p not in sys.path:
            sys.path.insert(0, p)
    import concourse.bacc as bacc_mod

import ml_dtypes
import concourse.bass as bass
import concourse.tile as tile
from concourse import mybir
from concourse.bass_utils import run_bass_kernel_spmd

F32 = mybir.dt.float32
BF16 = mybir.dt.bfloat16
I16 = mybir.dt.int16
BF = ml_dtypes.bfloat16

CORES = 8
NEG_ATT = 0.2
NEG_OUT = 0.01
class Cfg:
    def __init__(self, N, E, SPLIT, NBLK, F=128, H=4, C=32, ED=16, K=None):
        assert NBLK % CORES == 0 and SPLIT % 128 == 0
        self.N, self.E, self.SPLIT, self.NBLK = N, E, SPLIT, NBLK
        self.F, self.H, self.C, self.ED = F, H, C, ED
        self.HC = H * C
        self.NPAD = NBLK * 128
        self.ABLK = SPLIT // 128
        self.BBLK = NBLK - self.ABLK
        assert self.ABLK % CORES == 0 and self.BBLK % CORES == 0
        self.ASLOTS = self.ABLK // CORES
        self.BSLOTS = self.BBLK // CORES
        self.BPC = self.ASLOTS + self.BSLOTS
        if K is None:
            K = next(k for k in (5, 4, 3, 2, 1) if self.BPC % k == 0)
        self.K = K
        assert self.BPC % K == 0
        self.NSG = self.BPC // K
        self.nA = None
        self.nB = None

    @property
    def G(self):
        return self.nA + self.nB


def full_cfg():
    # NBLK=400 -> NPAD=51200 >= 50000, ABLK=256, BBLK=144 (18/core), BPC=50.
    return Cfg(N=50000, E=800000, SPLIT=32768, NBLK=400)


def _wrap_idx(vals):
    """int16 index list -> [128, len/16] wrapped layout for dma_gather."""
    m = vals.shape[-1]
    assert m % 16 == 0
    base = vals.reshape(*vals.shape[:-1], m // 16, 16)
    base = np.swapaxes(base, -1, -2)  # [..., 16, m/16]
    return np.concatenate([base] * 8, axis=-2).astype(np.int16)  # [...,128,m/16]


def prep(cfg, node_features, edge_index, edge_attr, W, W_edge,
         att_src, att_dst, att_edge, bias):
    N, E, H, C, ED, F = cfg.N, cfg.E, cfg.H, cfg.C, cfg.ED, cfg.F
    NPAD, NBLK, SPLIT = cfg.NPAD, cfg.NBLK, cfg.SPLIT
    K = cfg.K

    src = np.asarray(edge_index[0], np.int64)
    dst = np.asarray(edge_index[1], np.int64)
    ea = np.asarray(edge_attr, np.float32)

    # ---- fold weights ----
    V = np.einsum("dhc,hc->dh", np.asarray(W_edge, np.float32).reshape(ED, H, C),
                  np.asarray(att_edge, np.float32))  # [ED, H]

    # ---- LPT bin-pack nodes into blocks by in-degree ----
    deg = np.bincount(dst, minlength=N)
    order = np.argsort(-deg, kind="stable")
    import heapq
    heap = [(0, b) for b in range(NBLK)]
    heapq.heapify(heap)
    counts = np.zeros(NBLK, np.int64)
    blk_of = np.empty(N, np.int64)
    for n in order:
        load, b = heapq.heappop(heap)
        blk_of[n] = b
        counts[b] += 1
        if counts[b] < 128:
            heapq.heappush(heap, (int(load) + int(deg[n]), b))
    prow = np.empty(N, np.int64)
    pos_ctr = np.zeros(NBLK, np.int64)
    for n in np.argsort(blk_of, kind="stable"):
        b = blk_of[n]
        prow[n] = b * 128 + pos_ctr[b]
        pos_ctr[b] += 1

    srow = prow[src]
    drow = prow[dst]
    eblk = drow // 128
    half = (srow >= SPLIT).astype(np.int64)

    key = eblk * 2 + half
    eorder = np.argsort(key, kind="stable")
    ks = key[eorder]
    counts2 = np.bincount(ks, minlength=NBLK * 2)
    starts = np.zeros(NBLK * 2, np.int64)
    starts[1:] = np.cumsum(counts2)[:-1]
    rank = np.arange(E) - starts[ks]

    LA = counts2[0::2]
    LB = counts2[1::2]
    nA = max(1, int(np.ceil(LA.max() / 128)))
    nB = max(1, int(np.ceil(LB.max() / 128)))
    cfg.nA, cfg.nB = nA, nB
    G = cfg.G

    # ---- grids ----
    gidxA = np.zeros((NBLK, nA * 128), np.int64)
    gidxB = np.zeros((NBLK, nB * 128), np.int64)
    gdrt = np.full((NBLK, 128, G), 128.0, np.float32)
    gdrr = np.full((NBLK, G, 128), 128.0, np.float32)
    geao = np.zeros((NBLK, 128, G, ED + 1), np.float32)

    es, eh, er = eblk[eorder], half[eorder], rank
    esrow, edrow = srow[eorder], drow[eorder]
    eea = ea[eorder]

    a_m = eh == 0
    gidxA[es[a_m], er[a_m]] = esrow[a_m]
    b_m = ~a_m
    gidxB[es[b_m], er[b_m]] = esrow[b_m] - SPLIT

    gchunk = np.where(a_m, er // 128, nA + er // 128)
    gj = er % 128
    drel = (edrow % 128).astype(np.float32)
    gdrt[es, gj, gchunk] = drel
    gdrr[es, gchunk, gj] = drel
    geao[es, gj, gchunk, :ED] = eea
    geao[es, gj, gchunk, ED] = 1.0

    gidxS = np.empty((NBLK, 128), np.int64)
    gidxS[:] = np.arange(NBLK)[:, None] * 128 + np.arange(128)[None, :]
    gidxS[cfg.ABLK:] -= SPLIT

    # ---- per-core block assignment ----
    AS, BS = cfg.ASLOTS, cfg.BSLOTS
    core_blocks = []
    for c in range(CORES):
        abl = list(range(c * AS, (c + 1) * AS))
        bbl = list(range(cfg.ABLK + c * BS, cfg.ABLK + (c + 1) * BS))
        core_blocks.append(abl + bbl)
    core_blocks = np.array(core_blocks)  # [CORES, BPC]

    # ---- node feature table (permuted, transposed, bf16) ----
    nfT = np.zeros((F, NPAD), np.float32)
    nf = np.asarray(node_features, np.float32)
    inv = np.full(NPAD, -1, np.int64)
    inv[prow] = np.arange(N)
    valid = inv >= 0
    nfT[:, valid] = nf[inv[valid]].T

    att_src_rep = np.tile(np.asarray(att_src, np.float32)[None], (128, 1, 1))
    att_dst_rep = np.tile(np.asarray(att_dst, np.float32)[None], (128, 1, 1))
    V_rep = np.tile(V.T[None], (128, 1, 1))  # [128, H, ED]
    bias_rep = np.tile(np.asarray(bias, np.float32)[None], (128, 1))
    iota_row = np.tile(np.arange(128, dtype=np.float32)[None], (128, 1))
    iota_col = np.arange(128, dtype=np.float32)[:, None]  # [128,1]
    ones_row = np.ones((1, 128), np.float32)

    NSG = cfg.NSG
    in_maps = []
    for c in range(CORES):
        bl = core_blocks[c]
        # super-group index/grid consolidation
        iA = gidxA[bl].reshape(NSG, K * nA * 128)
        iB = gidxB[bl].reshape(NSG, K * nB * 128)
        sA = gidxS[bl[:AS]].reshape(-1)
        sB = gidxS[bl[AS:]].reshape(-1)
        in_maps.append({
            "nfT": nfT.astype(BF),
            "Wb": np.asarray(W, np.float32).astype(BF),
            "idxA": _wrap_idx(iA),
            "idxB": _wrap_idx(iB),
            "idxSA": _wrap_idx(sA[None])[0],
            "idxSB": _wrap_idx(sB[None])[0],
            "dstrelT": gdrt[bl].reshape(NSG, K, 128, G).transpose(0, 2, 1, 3)
                        .reshape(NSG, 128, K * G).astype(BF).copy(),
            "dstrelR": gdrr[bl].reshape(NSG, K * G * 128).astype(BF),
            "eaones": geao[bl].astype(BF),
            "att_src_rep": att_src_rep.astype(BF),
            "att_dst_rep": att_dst_rep.astype(BF),
            "V_rep": V_rep.astype(BF),
            "bias_rep": bias_rep,
            "iota_row": iota_row.astype(BF),
            "iota_col": iota_col.astype(BF),
            "ones_row": ones_row.astype(BF),
        })

    meta = dict(prow=prow, core_blocks=core_blocks, valid=valid, inv=inv)
    return in_maps, meta


def vw(ap, pairs, extra_offset=0):
    """Manual AP view: keep tensor, adjust offset, replace ap pairs."""
    return bass.AP(tensor=ap.tensor, offset=ap.offset + extra_offset, ap=pairs)


def build(cfg):
    NPAD, NBLK, SPLIT = cfg.NPAD, cfg.NBLK, cfg.SPLIT
    BPC, AS, G, nA, nB = cfg.BPC, cfg.ASLOTS, cfg.G, cfg.nA, cfg.nB
    NSG, K = cfg.NSG, cfg.K
    F, H, C, ED = cfg.F, cfg.H, cfg.C, cfg.ED
    HC = cfg.HC
    EW = ED + 1
    RW = HC + H + EW  # 149
    NX4 = NPAD // 512

    nc = bacc_mod.Bacc(num_swdge_queues=4)

    nfT = nc.dram_tensor("nfT", [F, NPAD], BF16, kind="ExternalInput")
    Wb = nc.dram_tensor("Wb", [F, HC], BF16, kind="ExternalInput")
    idxA = nc.dram_tensor("idxA", [NSG, 128, K * nA * 8], I16, kind="ExternalInput")
    idxB = nc.dram_tensor("idxB", [NSG, 128, K * nB * 8], I16, kind="ExternalInput")
    idxSA = nc.dram_tensor("idxSA", [128, AS * 8], I16, kind="ExternalInput")
    idxSB = nc.dram_tensor("idxSB", [128, (BPC - AS) * 8], I16, kind="ExternalInput")
    dstrelT = nc.dram_tensor("dstrelT", [NSG, 128, K * G], BF16, kind="ExternalInput")
    dstrelR = nc.dram_tensor("dstrelR", [NSG, K * G * 128], BF16, kind="ExternalInput")
    eaones = nc.dram_tensor("eaones", [BPC, 128, G, EW], BF16, kind="ExternalInput")
    att_src_rep = nc.dram_tensor("att_src_rep", [128, H, C], BF16, kind="ExternalInput")
    att_dst_rep = nc.dram_tensor("att_dst_rep", [128, H, C], BF16, kind="ExternalInput")
    V_rep = nc.dram_tensor("V_rep", [128, H, ED], BF16, kind="ExternalInput")
    bias_rep = nc.dram_tensor("bias_rep", [128, HC], F32, kind="ExternalInput")
    iota_row = nc.dram_tensor("iota_row", [128, 128], BF16, kind="ExternalInput")
    iota_col = nc.dram_tensor("iota_col", [128, 1], BF16, kind="ExternalInput")
    ones_row = nc.dram_tensor("ones_row", [1, 128], BF16, kind="ExternalInput")
    out = nc.dram_tensor("out", [BPC, 128, HC], F32, kind="ExternalOutput")

    with tile.TileContext(nc) as tc:
        with (
            tc.tile_pool(name="dram", bufs=1, space="DRAM") as dpool,
            tc.tile_pool(name="const", bufs=1) as cpool,
            tc.tile_pool(name="p1", bufs=3) as p1pool,
            tc.tile_pool(name="p1ps", bufs=2, space="PSUM") as p1ps,
            tc.tile_pool(name="xgp", bufs=2) as xgp,
            tc.tile_pool(name="sgp", bufs=2) as sgp,
            tc.tile_pool(name="big", bufs=2) as bpool,
            tc.tile_pool(name="med", bufs=2) as mpool,
            tc.tile_pool(name="sml", bufs=3) as spool,
            tc.tile_pool(name="msk", bufs=3) as kpool,
            tc.tile_pool(name="psA", bufs=2, space="PSUM") as psA,
            tc.tile_pool(name="psB", bufs=1, space="PSUM") as psB,
            tc.tile_pool(name="psT", bufs=2, space="PSUM") as psT,
        ):
            xt = dpool.tile([NPAD, HC], BF16)

            # constants
            c_w = cpool.tile([F, HC], BF16)
            nc.sync.dma_start(out=c_w[:], in_=Wb[:])
            c_asrc = cpool.tile([128, H, C], BF16)
            nc.sync.dma_start(out=c_asrc[:], in_=att_src_rep[:])
            c_adst = cpool.tile([128, H, C], BF16)
            nc.sync.dma_start(out=c_adst[:], in_=att_dst_rep[:])
            c_v = cpool.tile([128, H, ED], BF16)
            nc.sync.dma_start(out=c_v[:], in_=V_rep[:])
            c_bias = cpool.tile([128, HC], F32)
            nc.sync.dma_start(out=c_bias[:], in_=bias_rep[:])
            c_iota = cpool.tile([128, 128], BF16)
            nc.sync.dma_start(out=c_iota[:], in_=iota_row[:])
            c_iotc = cpool.tile([128, 1], BF16)
            nc.sync.dma_start(out=c_iotc[:], in_=iota_col[:])
            c_ones = cpool.tile([1, 128], BF16)
            nc.sync.dma_start(out=c_ones[:], in_=ones_row[:])

            # ---- Phase 1: xt = (nf @ W) in bf16, 512 nodes per iteration ----
            for k in range(NX4):
                lt = p1pool.tile([F, 512], BF16, tag="p1lhs")
                nc.sync.dma_start(out=lt[:], in_=nfT[:, k * 512:(k + 1) * 512])
                ps = p1ps.tile([128, 4, HC], F32, tag="p1ps")
                for c4 in range(4):
                    nc.tensor.matmul(out=ps[:, c4, :],
                                     lhsT=lt[:, c4 * 128:(c4 + 1) * 128],
                                     rhs=c_w[:], start=True, stop=True)
                xb = p1pool.tile([128, 4, HC], BF16, tag="p1out")
                nc.scalar.copy(out=xb[:], in_=ps[:])
                xt_v = vw(xt[:], [[HC, 128], [128 * HC, 4], [1, HC]],
                          extra_offset=k * 512 * HC)
                nc.sync.dma_start(out=xt_v, in_=xb[:])

            # ---- self-row gathers (all blocks at once) ----
            isa = sgp.tile([128, AS * 8], I16, tag="isa")
            nc.sync.dma_start(out=isa[:], in_=idxSA[:])
            isb = sgp.tile([128, (BPC - AS) * 8], I16, tag="isb")
            nc.sync.dma_start(out=isb[:], in_=idxSB[:])
            xs_sup = sgp.tile([128, BPC, HC], BF16, tag="xs")
            nc.gpsimd.dma_gather(
                xs_sup[:, 0:AS, :], xt[:], isa[:], AS * 128, AS * 128, HC,
                queue_num=0, single_packet=False)
            nc.gpsimd.dma_gather(
                xs_sup[:, AS:BPC, :], xt[SPLIT:, :], isb[:],
                (BPC - AS) * 128, (BPC - AS) * 128, HC,
                queue_num=1, single_packet=False)

            # ---- Phase 2 ----
            for sg in range(NSG):
                ia = spool.tile([128, K * nA * 8], I16, tag="ia")
                nc.sync.dma_start(out=ia[:], in_=idxA[sg])
                ib = spool.tile([128, K * nB * 8], I16, tag="ib")
                nc.sync.dma_start(out=ib[:], in_=idxB[sg])
                drt = spool.tile([128, K * G], BF16, tag="drt")
                nc.sync.dma_start(out=drt[:], in_=dstrelT[sg])
                drr = spool.tile([1, K * G * 128], BF16, tag="drr")
                nc.sync.dma_start(out=drr[:], in_=dstrelR[sg])

                # xg layout: [A-chunks of K blocks | B-chunks of K blocks]
                xg = xgp.tile([128, K * G, HC], BF16, tag="xg")
                nc.gpsimd.dma_gather(
                    xg[:, 0:K * nA, :],
                    xt[:], ia[:], K * nA * 128, K * nA * 128, HC,
                    queue_num=(2 * sg) % 4, single_packet=False)
                nc.gpsimd.dma_gather(
                    xg[:, K * nA:K * G, :],
                    xt[SPLIT:, :], ib[:], K * nB * 128, K * nB * 128, HC,
                    queue_num=(2 * sg + 1) % 4, single_packet=False)

                for bb in range(K):
                    b = sg * K + bb
                    part = xg[:].ap[0]
                    xg4A = vw(xg[:], [part, [HC, nA], [C, H], [1, C]],
                              extra_offset=bb * nA * HC)
                    xg4B = vw(xg[:], [part, [HC, nB], [C, H], [1, C]],
                              extra_offset=(K * nA + bb * nB) * HC)

                    rhs = bpool.tile([128, G, RW], BF16, tag="rhs")
                    nc.sync.dma_start(out=rhs[:, :, HC + H:RW], in_=eaones[b])

                    # --- maskT [j, G, i] ---
                    maskT = bpool.tile([128, G, 128], BF16, tag="maskT")
                    drt_b = vw(drt[:], [drt[:].ap[0], [1, G], [0, 128]],
                               extra_offset=bb * G)
                    iota_b = vw(c_iota[:], [c_iota[:].ap[0], [0, G], [1, 128]])
                    nc.vector.tensor_tensor(out=maskT[:], in0=drt_b, in1=iota_b,
                                            op=mybir.AluOpType.is_equal)

                    # --- a_src_e [j, G, H] ---
                    t_as = bpool.tile([128, G, H, C], BF16, tag="t_as")
                    asrc_bA = vw(c_asrc[:], [c_asrc[:].ap[0], [0, nA], [C, H], [1, C]])
                    asrc_bB = vw(c_asrc[:], [c_asrc[:].ap[0], [0, nB], [C, H], [1, C]])
                    nc.vector.tensor_tensor(out=t_as[:, 0:nA], in0=xg4A,
                                            in1=asrc_bA, op=mybir.AluOpType.mult)
                    nc.vector.tensor_tensor(out=t_as[:, nA:G], in0=xg4B,
                                            in1=asrc_bB, op=mybir.AluOpType.mult)
                    a_src_e = mpool.tile([128, G, H], F32, tag="a_src_e")
                    nc.vector.reduce_sum(out=a_src_e[:], in_=t_as[:],
                                         axis=mybir.AxisListType.X)

                    # --- a_e [j, G, H] ---
                    t_ae = mpool.tile([128, G, H, ED], BF16, tag="t_ae")
                    ea_b = vw(rhs[:], [rhs[:].ap[0], [RW, G], [0, H], [1, ED]],
                              extra_offset=HC + H)
                    v_b = vw(c_v[:], [c_v[:].ap[0], [0, G], [ED, H], [1, ED]])
                    nc.vector.tensor_tensor(out=t_ae[:], in0=ea_b, in1=v_b,
                                            op=mybir.AluOpType.mult)
                    a_e = mpool.tile([128, G, H], F32, tag="a_e")
                    nc.vector.reduce_sum(out=a_e[:], in_=t_ae[:],
                                         axis=mybir.AxisListType.X)

                    # --- per-node a_dst/a_src (self rows) ---
                    t_bs = mpool.tile([128, 2, H, C], BF16, tag="t_bs")
                    xsb = vw(xs_sup[:], [xs_sup[:].ap[0], [0, 1], [C, H], [1, C]],
                             extra_offset=b * HC)
                    ad2 = vw(c_adst[:], [c_adst[:].ap[0], [0, 1], [C, H], [1, C]])
                    as2 = vw(c_asrc[:], [c_asrc[:].ap[0], [0, 1], [C, H], [1, C]])
                    nc.vector.tensor_tensor(out=t_bs[:, 0:1], in0=xsb, in1=ad2,
                                            op=mybir.AluOpType.mult)
                    nc.vector.tensor_tensor(out=t_bs[:, 1:2], in0=xsb, in1=as2,
                                            op=mybir.AluOpType.mult)
                    blkv = spool.tile([128, 2, H], F32, tag="blkv")
                    nc.vector.reduce_sum(out=blkv[:], in_=t_bs[:],
                                         axis=mybir.AxisListType.X)
                    a_dst_bf = spool.tile([128, H], BF16, tag="a_dst_bf")
                    nc.vector.tensor_copy(out=a_dst_bf[:], in_=blkv[:, 0, :])

                    # --- a_dst expansion: bcast matmul + is_equal + matmul ---
                    ps3 = psB.tile([128, G, H], F32, tag="ps3")
                    ngrp = (G + 3) // 4
                    for grp in range(ngrp):
                        g0 = grp * 4
                        gw = min(4, G - g0)
                        psb_t = psT.tile([128, 4, 128], F32, tag="psb")
                        drr_s = vw(drr[:], [drr[:].ap[0], [1, gw * 128]],
                                   extra_offset=(bb * G + g0) * 128)
                        nc.tensor.matmul(
                            out=vw(psb_t[:], [psb_t[:].ap[0], [1, gw * 128]]),
                            lhsT=c_ones[:], rhs=drr_s, start=True, stop=True)
                        mask4 = kpool.tile([128, 4, 128], BF16, tag="mask4")
                        iotc_b = vw(c_iotc[:], [c_iotc[:].ap[0], [0, gw], [0, 128]])
                        nc.vector.tensor_tensor(
                            out=mask4[:, 0:gw], in0=psb_t[:, 0:gw], in1=iotc_b,
                            op=mybir.AluOpType.is_equal)
                        for gg in range(gw):
                            nc.tensor.matmul(out=ps3[:, g0 + gg, :],
                                             lhsT=mask4[:, gg, :],
                                             rhs=a_dst_bf[:],
                                             start=True, stop=True)

                    # --- alpha / ex ---
                    alpha = mpool.tile([128, G, H], F32, tag="alpha")
                    nc.vector.tensor_add(out=alpha[:], in0=ps3[:], in1=a_src_e[:])
                    nc.vector.tensor_add(out=alpha[:], in0=alpha[:], in1=a_e[:])
                    lrel = mpool.tile([128, G, H], F32, tag="lrel")
                    nc.vector.tensor_scalar_mul(out=lrel[:], in0=alpha[:],
                                                scalar1=NEG_ATT)
                    nc.vector.tensor_tensor(out=lrel[:], in0=lrel[:], in1=alpha[:],
                                            op=mybir.AluOpType.max)
                    nc.scalar.activation(out=rhs[:, :, HC:HC + H], in_=lrel[:],
                                         func=mybir.ActivationFunctionType.Exp)

                    # --- wmsg ---
                    ex_bA = vw(rhs[:], [rhs[:].ap[0], [RW, nA], [1, H], [0, C]],
                               extra_offset=HC)
                    ex_bB = vw(rhs[:], [rhs[:].ap[0], [RW, nB], [1, H], [0, C]],
                               extra_offset=nA * RW + HC)
                    woutA = vw(rhs[:], [rhs[:].ap[0], [RW, nA], [C, H], [1, C]])
                    woutB = vw(rhs[:], [rhs[:].ap[0], [RW, nB], [C, H], [1, C]],
                               extra_offset=nA * RW)
                    nc.vector.tensor_tensor(out=woutA, in0=xg4A, in1=ex_bA,
                                            op=mybir.AluOpType.mult)
                    nc.vector.tensor_tensor(out=woutB, in0=xg4B, in1=ex_bB,
                                            op=mybir.AluOpType.mult)

                    # --- main accumulation ---
                    psm = psA.tile([128, RW], F32, tag="psm")
                    for g in range(G):
                        nc.tensor.matmul(out=psm[:], lhsT=maskT[:, g, :],
                                         rhs=rhs[:, g, :],
                                         start=(g == 0), stop=(g == G - 1))

                    # --- self loop + normalize ---
                    cntc = spool.tile([128, 1], F32, tag="cntc")
                    nc.vector.tensor_scalar_max(out=cntc[:], in0=psm[:, RW - 1:RW],
                                                scalar1=1.0)
                    rcnt = spool.tile([128, 1], F32, tag="rcnt")
                    nc.vector.reciprocal(out=rcnt[:], in_=cntc[:])
                    lattr = spool.tile([128, ED], F32, tag="lattr")
                    nc.vector.tensor_scalar_mul(out=lattr[:],
                                                in0=psm[:, HC + H:HC + H + ED],
                                                scalar1=rcnt[:, 0:1])
                    t_al = spool.tile([128, H, ED], F32, tag="t_al")
                    lattr_b = vw(lattr[:], [lattr[:].ap[0], [0, H], [1, ED]])
                    v_b2 = vw(c_v[:], [c_v[:].ap[0], [ED, H], [1, ED]])
                    nc.vector.tensor_tensor(out=t_al[:], in0=lattr_b, in1=v_b2,
                                            op=mybir.AluOpType.mult)
                    a_el = spool.tile([128, H], F32, tag="a_el")
                    nc.vector.reduce_sum(out=a_el[:], in_=t_al[:],
                                         axis=mybir.AxisListType.X)
                    alf = spool.tile([128, H], F32, tag="alf")
                    nc.vector.tensor_add(out=alf[:], in0=blkv[:, 0, :],
                                         in1=blkv[:, 1, :])
                    nc.vector.tensor_add(out=alf[:], in0=alf[:], in1=a_el[:])
                    alf2 = spool.tile([128, H], F32, tag="alf2")
                    nc.vector.tensor_scalar_mul(out=alf2[:], in0=alf[:],
                                                scalar1=NEG_ATT)
                    nc.vector.tensor_tensor(out=alf2[:], in0=alf2[:], in1=alf[:],
                                            op=mybir.AluOpType.max)
                    exl = spool.tile([128, H], F32, tag="exl")
                    nc.scalar.activation(out=exl[:], in_=alf2[:],
                                         func=mybir.ActivationFunctionType.Exp)

                    den = spool.tile([128, H], F32, tag="den")
                    nc.vector.tensor_add(out=den[:], in0=psm[:, HC:HC + H],
                                         in1=exl[:])
                    rden = spool.tile([128, H], F32, tag="rden")
                    nc.vector.reciprocal(out=rden[:], in_=den[:])

                    smsg = mpool.tile([128, HC], F32, tag="smsg")
                    exl_b = vw(exl[:], [exl[:].ap[0], [1, H], [0, C]])
                    xs2 = vw(xs_sup[:], [xs_sup[:].ap[0], [C, H], [1, C]],
                             extra_offset=b * HC)
                    nc.vector.tensor_tensor(out=smsg[:], in0=xs2, in1=exl_b,
                                            op=mybir.AluOpType.mult)
                    agg = mpool.tile([128, HC], F32, tag="agg")
                    nc.vector.tensor_add(out=agg[:], in0=psm[:, 0:HC], in1=smsg[:])
                    rden_b = vw(rden[:], [rden[:].ap[0], [1, H], [0, C]])
                    nc.vector.tensor_tensor(out=agg[:], in0=agg[:], in1=rden_b,
                                            op=mybir.AluOpType.mult)
                    nc.vector.tensor_add(out=agg[:], in0=agg[:], in1=c_bias[:])
                    osb = mpool.tile([128, HC], F32, tag="osb")
                    nc.scalar.mul(out=osb[:], in_=agg[:], mul=NEG_OUT)
                    nc.vector.tensor_tensor(out=osb[:], in0=osb[:], in1=agg[:],
                                            op=mybir.AluOpType.max)
                    nc.sync.dma_start(out=out[b], in_=osb[:])

    nc.finalize()
    return nc


def assemble(cfg, meta, results):
    """Gather per-core outputs back to full [N, HC] float32."""
    NPAD = cfg.NPAD
    flat = np.zeros((NPAD, cfg.HC), np.float32)
    for c in range(CORES):
        o = results[c]["out"]  # [BPC, 128, HC]
        bl = meta["core_blocks"][c]
        for s, b in enumerate(bl):
            flat[b * 128:(b + 1) * 128] = o[s]
    y = np.empty((cfg.N, cfg.HC), np.float32)
    y[meta["inv"][meta["valid"]]] = flat[meta["valid"]]
    return y


_BUILD_CACHE = {}


def kernel(**inputs):
    cfg = full_cfg()
    in_maps, meta = prep(cfg, **inputs)
    ckey = (cfg.N, cfg.E, cfg.nA, cfg.nB)
    if ckey in _BUILD_CACHE:
        nc = _BUILD_CACHE[ckey]
    else:
        nc = build(cfg)
        _BUILD_CACHE[ckey] = nc
    res = run_bass_kernel_spmd(nc, in_maps, core_ids=list(range(CORES)))
    return assemble(cfg, meta, res.results)


# revision 19
# speedup vs baseline: 1.4026x; 1.0584x over previous
"""GAT layer (gnn_message_passing) Trainium2 kernel, 8-core edge-parallel.

Strategy (dst-sorted edge-centric):
  - Host: fold attention weight vectors into small matrices; bin-pack nodes
    into NBLK blocks of 128 (LPT on in-degree) so every block has a near-equal
    number of incoming edges; sort edges by destination block; within a block
    split edges by gathered-row index (<32768 vs >=32768, the int16 limit of
    the dma_gather custom instruction) into two contiguous sub-lists.
  - Device (SPMD, 8 cores, NBLK/8 blocks each):
      Phase 1: x = node_features @ W projected once per core into a bf16
               DRAM table xt (256B rows).
      Phase 2: consolidated dma_gathers fetch edge source rows for groups of
               K blocks at a time; per block, one-hot destination masks are
               built with is_equal; the dst attention term is expanded per
               edge via a K=1 broadcast matmul + is_equal + small matmul;
               softmax runs without max-subtraction (logits are tiny); one
               fused PE matmul per 128-edge chunk accumulates messages,
               softmax denominators, self-loop edge_attr sums and counts.
               Self loops are handled analytically per block.
  - Host: un-permute rows of the gathered per-core outputs.
"""

import numpy as np

try:
    import concourse.bacc as bacc_mod  # noqa: F401
except Exception:  # pragma: no cover
    import sys

    for p in ("/opt/trn_rl_repo", "/root/.axon_site/_ro/trn_rl_repo"):
        if p not in sys.path:
            sys.path.insert(0, p)
    import concourse.bacc as bacc_mod

import ml_dtypes
import concourse.bass as bass
import concourse.tile as tile
from concourse import mybir
from concourse.bass_utils import run_bass_kernel_spmd

F32 = mybir.dt.float32
BF16 = mybir.dt.bfloat16
I16 = mybir.dt.int16
BF = ml_dtypes.bfloat16

CORES = 8
NEG_ATT = 0.2
NEG_OUT = 0.01
class Cfg:
    def __init__(self, N, E, SPLIT, NBLK, F=128, H=4, C=32, ED=16, K=None):
        assert NBLK % CORES == 0 and SPLIT % 128 == 0
        self.N, self.E, self.SPLIT, self.NBLK = N, E, SPLIT, NBLK
        self.F, self.H, self.C, self.ED = F, H, C, ED
        self.HC = H * C
        self.NPAD = NBLK * 128
        self.ABLK = SPLIT // 128
        self.BBLK = NBLK - self.ABLK
        assert self.ABLK % CORES == 0 and self.BBLK % CORES == 0
        self.ASLOTS = self.ABLK // CORES
        self.BSLOTS = self.BBLK // CORES
        self.BPC = self.ASLOTS + self.BSLOTS
        if K is None:
            K = next(k for k in (5, 4, 3, 2, 1) if self.BPC % k == 0)
        self.K = K
        assert self.BPC % K == 0
        self.NSG = self.BPC // K
        self.nA = None
        self.nB = None

    @property
    def G(self):
        return self.nA + self.nB


def full_cfg():
    # NBLK=400 -> NPAD=51200 >= 50000, ABLK=256, BBLK=144 (18/core), BPC=50.
    return Cfg(N=50000, E=800000, SPLIT=32768, NBLK=400)


def _wrap_idx(vals):
    """int16 index list -> [128, len/16] wrapped layout for dma_gather."""
    m = vals.shape[-1]
    assert m % 16 == 0
    base = vals.reshape(*vals.shape[:-1], m // 16, 16)
    base = np.swapaxes(base, -1, -2)  # [..., 16, m/16]
    return np.concatenate([base] * 8, axis=-2).astype(np.int16)  # [...,128,m/16]


def prep(cfg, node_features, edge_index, edge_attr, W, W_edge,
         att_src, att_dst, att_edge, bias):
    N, E, H, C, ED, F = cfg.N, cfg.E, cfg.H, cfg.C, cfg.ED, cfg.F
    NPAD, NBLK, SPLIT = cfg.NPAD, cfg.NBLK, cfg.SPLIT
    K = cfg.K

    src = np.asarray(edge_index[0], np.int64)
    dst = np.asarray(edge_index[1], np.int64)
    ea = np.asarray(edge_attr, np.float32)

    # ---- fold weights ----
    V = np.einsum("dhc,hc->dh", np.asarray(W_edge, np.float32).reshape(ED, H, C),
                  np.asarray(att_edge, np.float32))  # [ED, H]

    # ---- LPT bin-pack nodes into blocks by in-degree ----
    deg = np.bincount(dst, minlength=N)
    order = np.argsort(-deg, kind="stable")
    import heapq
    heap = [(0, b) for b in range(NBLK)]
    heapq.heapify(heap)
    counts = np.zeros(NBLK, np.int64)
    blk_of = np.empty(N, np.int64)
    for n in order:
        load, b = heapq.heappop(heap)
        blk_of[n] = b
        counts[b] += 1
        if counts[b] < 128:
            heapq.heappush(heap, (int(load) + int(deg[n]), b))
    prow = np.empty(N, np.int64)
    pos_ctr = np.zeros(NBLK, np.int64)
    for n in np.argsort(blk_of, kind="stable"):
        b = blk_of[n]
        prow[n] = b * 128 + pos_ctr[b]
        pos_ctr[b] += 1

    srow = prow[src]
    drow = prow[dst]
    eblk = drow // 128
    half = (srow >= SPLIT).astype(np.int64)

    key = eblk * 2 + half
    eorder = np.argsort(key, kind="stable")
    ks = key[eorder]
    counts2 = np.bincount(ks, minlength=NBLK * 2)
    starts = np.zeros(NBLK * 2, np.int64)
    starts[1:] = np.cumsum(counts2)[:-1]
    rank = np.arange(E) - starts[ks]

    LA = counts2[0::2]
    LB = counts2[1::2]
    nA = max(1, int(np.ceil(LA.max() / 128)))
    nB = max(1, int(np.ceil(LB.max() / 128)))
    cfg.nA, cfg.nB = nA, nB
    G = cfg.G

    # ---- grids ----
    gidxA = np.zeros((NBLK, nA * 128), np.int64)
    gidxB = np.zeros((NBLK, nB * 128), np.int64)
    gdrt = np.full((NBLK, 128, G), 128.0, np.float32)
    gdrr = np.full((NBLK, G, 128), 128.0, np.float32)
    geao = np.zeros((NBLK, 128, G, ED + 1), np.float32)
    geaT = np.zeros((NBLK, G, ED, 128), np.float32)

    es, eh, er = eblk[eorder], half[eorder], rank
    esrow, edrow = srow[eorder], drow[eorder]
    eea = ea[eorder]

    a_m = eh == 0
    gidxA[es[a_m], er[a_m]] = esrow[a_m]
    b_m = ~a_m
    gidxB[es[b_m], er[b_m]] = esrow[b_m] - SPLIT

    gchunk = np.where(a_m, er // 128, nA + er // 128)
    gj = er % 128
    drel = (edrow % 128).astype(np.float32)
    gdrt[es, gj, gchunk] = drel
    gdrr[es, gchunk, gj] = drel
    geao[es, gj, gchunk, :ED] = eea
    geao[es, gj, gchunk, ED] = 1.0
    geaT[es, gchunk, :, gj] = eea

    gidxS = np.empty((NBLK, 128), np.int64)
    gidxS[:] = np.arange(NBLK)[:, None] * 128 + np.arange(128)[None, :]
    gidxS[cfg.ABLK:] -= SPLIT

    # ---- per-core block assignment ----
    AS, BS = cfg.ASLOTS, cfg.BSLOTS
    core_blocks = []
    for c in range(CORES):
        abl = list(range(c * AS, (c + 1) * AS))
        bbl = list(range(cfg.ABLK + c * BS, cfg.ABLK + (c + 1) * BS))
        core_blocks.append(abl + bbl)
    core_blocks = np.array(core_blocks)  # [CORES, BPC]

    # ---- node feature table (permuted, transposed, bf16) ----
    nfT = np.zeros((F, NPAD), np.float32)
    nf = np.asarray(node_features, np.float32)
    inv = np.full(NPAD, -1, np.int64)
    inv[prow] = np.arange(N)
    valid = inv >= 0
    nfT[:, valid] = nf[inv[valid]].T

    att_src_rep = np.tile(np.asarray(att_src, np.float32)[None], (128, 1, 1))
    att_dst_rep = np.tile(np.asarray(att_dst, np.float32)[None], (128, 1, 1))
    V_rep = np.tile(V.T[None], (128, 1, 1))  # [128, H, ED]
    bias_rep = np.tile(np.asarray(bias, np.float32)[None], (128, 1))
    iota_row = np.tile(np.arange(128, dtype=np.float32)[None], (128, 1))
    iota_col = np.arange(128, dtype=np.float32)[:, None]  # [128,1]
    ones_row = np.ones((1, 128), np.float32)

    NSG = cfg.NSG
    in_maps = []
    for c in range(CORES):
        bl = core_blocks[c]
        # super-group index/grid consolidation
        iA = gidxA[bl].reshape(NSG, K * nA * 128)
        iB = gidxB[bl].reshape(NSG, K * nB * 128)
        sA = gidxS[bl[:AS]].reshape(-1)
        sB = gidxS[bl[AS:]].reshape(-1)
        in_maps.append({
            "nfT": nfT.astype(BF),
            "Wb": np.asarray(W, np.float32).astype(BF),
            "idxA": _wrap_idx(iA),
            "idxB": _wrap_idx(iB),
            "idxSA": _wrap_idx(sA[None])[0],
            "idxSB": _wrap_idx(sB[None])[0],
            "dstrelT": gdrt[bl].reshape(NSG, K, 128, G).transpose(0, 2, 1, 3)
                        .reshape(NSG, 128, K * G).astype(BF).copy(),
            "dstrelR": gdrr[bl].reshape(NSG, K * G * 128).astype(BF),
            "eaones": geao[bl].astype(BF),
            "eaT": geaT[bl].reshape(NSG, K, G, ED, 128).transpose(0, 3, 1, 2, 4)
                     .reshape(NSG, ED, K * G, 128).astype(BF).copy(),
            "V16": V.astype(BF),
            "att_src_rep": att_src_rep.astype(BF),
            "att_dst_rep": att_dst_rep.astype(BF),
            "V_rep": V_rep.astype(BF),
            "bias_rep": bias_rep,
            "iota_row": iota_row.astype(BF),
            "iota_col": iota_col.astype(BF),
            "ones_row": ones_row.astype(BF),
        })

    meta = dict(prow=prow, core_blocks=core_blocks, valid=valid, inv=inv)
    return in_maps, meta


def vw(ap, pairs, extra_offset=0):
    """Manual AP view: keep tensor, adjust offset, replace ap pairs."""
    return bass.AP(tensor=ap.tensor, offset=ap.offset + extra_offset, ap=pairs)


def build(cfg):
    NPAD, NBLK, SPLIT = cfg.NPAD, cfg.NBLK, cfg.SPLIT
    BPC, AS, G, nA, nB = cfg.BPC, cfg.ASLOTS, cfg.G, cfg.nA, cfg.nB
    NSG, K = cfg.NSG, cfg.K
    F, H, C, ED = cfg.F, cfg.H, cfg.C, cfg.ED
    HC = cfg.HC
    EW = ED + 1
    RW = HC + H + EW  # 149
    NX4 = NPAD // 512

    nc = bacc_mod.Bacc(num_swdge_queues=4)

    nfT = nc.dram_tensor("nfT", [F, NPAD], BF16, kind="ExternalInput")
    Wb = nc.dram_tensor("Wb", [F, HC], BF16, kind="ExternalInput")
    idxA = nc.dram_tensor("idxA", [NSG, 128, K * nA * 8], I16, kind="ExternalInput")
    idxB = nc.dram_tensor("idxB", [NSG, 128, K * nB * 8], I16, kind="ExternalInput")
    idxSA = nc.dram_tensor("idxSA", [128, AS * 8], I16, kind="ExternalInput")
    idxSB = nc.dram_tensor("idxSB", [128, (BPC - AS) * 8], I16, kind="ExternalInput")
    dstrelT = nc.dram_tensor("dstrelT", [NSG, 128, K * G], BF16, kind="ExternalInput")
    dstrelR = nc.dram_tensor("dstrelR", [NSG, K * G * 128], BF16, kind="ExternalInput")
    eaones = nc.dram_tensor("eaones", [BPC, 128, G, EW], BF16, kind="ExternalInput")
    eaT = nc.dram_tensor("eaT", [NSG, ED, K * G, 128], BF16, kind="ExternalInput")
    V16 = nc.dram_tensor("V16", [ED, H], BF16, kind="ExternalInput")
    att_src_rep = nc.dram_tensor("att_src_rep", [128, H, C], BF16, kind="ExternalInput")
    att_dst_rep = nc.dram_tensor("att_dst_rep", [128, H, C], BF16, kind="ExternalInput")
    V_rep = nc.dram_tensor("V_rep", [128, H, ED], BF16, kind="ExternalInput")
    bias_rep = nc.dram_tensor("bias_rep", [128, HC], F32, kind="ExternalInput")
    iota_row = nc.dram_tensor("iota_row", [128, 128], BF16, kind="ExternalInput")
    iota_col = nc.dram_tensor("iota_col", [128, 1], BF16, kind="ExternalInput")
    ones_row = nc.dram_tensor("ones_row", [1, 128], BF16, kind="ExternalInput")
    out = nc.dram_tensor("out", [BPC, 128, HC], F32, kind="ExternalOutput")

    with tile.TileContext(nc) as tc:
        with (
            tc.tile_pool(name="dram", bufs=1, space="DRAM") as dpool,
            tc.tile_pool(name="const", bufs=1) as cpool,
            tc.tile_pool(name="p1", bufs=3) as p1pool,
            tc.tile_pool(name="p1ps", bufs=2, space="PSUM") as p1ps,
            tc.tile_pool(name="xgp", bufs=2) as xgp,
            tc.tile_pool(name="sgp", bufs=2) as sgp,
            tc.tile_pool(name="big", bufs=2) as bpool,
            tc.tile_pool(name="med", bufs=2) as mpool,
            tc.tile_pool(name="sml", bufs=3) as spool,
            tc.tile_pool(name="msk", bufs=3) as kpool,
            tc.tile_pool(name="psA", bufs=2, space="PSUM") as psA,
            tc.tile_pool(name="psB", bufs=1, space="PSUM") as psB,
            tc.tile_pool(name="psT", bufs=2, space="PSUM") as psT,
        ):
            xtA = dpool.tile([SPLIT, HC], BF16)
            xtB = dpool.tile([NPAD - SPLIT, HC], BF16)

            # constants
            c_w = cpool.tile([F, HC], BF16)
            nc.sync.dma_start(out=c_w[:], in_=Wb[:])
            c_asrc = cpool.tile([128, H, C], BF16)
            nc.sync.dma_start(out=c_asrc[:], in_=att_src_rep[:])
            c_adst = cpool.tile([128, H, C], BF16)
            nc.sync.dma_start(out=c_adst[:], in_=att_dst_rep[:])
            c_v = cpool.tile([128, H, ED], BF16)
            nc.sync.dma_start(out=c_v[:], in_=V_rep[:])
            c_bias = cpool.tile([128, HC], F32)
            nc.sync.dma_start(out=c_bias[:], in_=bias_rep[:])
            c_iota = cpool.tile([128, 128], BF16)
            nc.sync.dma_start(out=c_iota[:], in_=iota_row[:])
            c_iotc = cpool.tile([128, 1], BF16)
            nc.sync.dma_start(out=c_iotc[:], in_=iota_col[:])
            c_ones = cpool.tile([1, 128], BF16)
            nc.sync.dma_start(out=c_ones[:], in_=ones_row[:])
            c_v16 = cpool.tile([ED, H], BF16)
            nc.sync.dma_start(out=c_v16[:], in_=V16[:])

            # ---- Phase 1: xt = (nf @ W) in bf16, 512 nodes per iteration ----
            NSPL = SPLIT // 512
            for k in range(NX4):
                lt = p1pool.tile([F, 512], BF16, tag="p1lhs")
                nc.sync.dma_start(out=lt[:], in_=nfT[:, k * 512:(k + 1) * 512])
                ps = p1ps.tile([128, 4, HC], F32, tag="p1ps")
                for c4 in range(4):
                    nc.tensor.matmul(out=ps[:, c4, :],
                                     lhsT=lt[:, c4 * 128:(c4 + 1) * 128],
                                     rhs=c_w[:], start=True, stop=True)
                xb = p1pool.tile([128, 4, HC], BF16, tag="p1out")
                nc.scalar.copy(out=xb[:], in_=ps[:])
                xtt = xtA if k < NSPL else xtB
                koff = k if k < NSPL else k - NSPL
                xt_v = vw(xtt[:], [[HC, 128], [128 * HC, 4], [1, HC]],
                          extra_offset=koff * 512 * HC)
                nc.sync.dma_start(out=xt_v, in_=xb[:])

            # ---- self-row gathers (all blocks at once) ----
            isa = sgp.tile([128, AS * 8], I16, tag="isa")
            nc.sync.dma_start(out=isa[:], in_=idxSA[:])
            isb = sgp.tile([128, (BPC - AS) * 8], I16, tag="isb")
            nc.sync.dma_start(out=isb[:], in_=idxSB[:])
            xs_sup = sgp.tile([128, BPC, HC], BF16, tag="xs")
            nc.gpsimd.dma_gather(
                xs_sup[:, 0:AS, :], xtA[:], isa[:], AS * 128, AS * 128, HC,
                queue_num=0, single_packet=False)
            nc.gpsimd.dma_gather(
                xs_sup[:, AS:BPC, :], xtB[:], isb[:],
                (BPC - AS) * 128, (BPC - AS) * 128, HC,
                queue_num=1, single_packet=False)

            # ---- Phase 2 ----
            for sg in range(NSG):
                ia = spool.tile([128, K * nA * 8], I16, tag="ia")
                nc.sync.dma_start(out=ia[:], in_=idxA[sg])
                ib = spool.tile([128, K * nB * 8], I16, tag="ib")
                nc.sync.dma_start(out=ib[:], in_=idxB[sg])
                drt = spool.tile([128, K * G], BF16, tag="drt")
                nc.sync.dma_start(out=drt[:], in_=dstrelT[sg])


                # xg layout: [A-chunks of K blocks | B-chunks of K blocks]
                xg = xgp.tile([128, K * G, HC], BF16, tag="xg")
                nc.gpsimd.dma_gather(
                    xg[:, 0:K * nA, :],
                    xtA[:], ia[:], K * nA * 128, K * nA * 128, HC,
                    queue_num=(2 * sg) % 4, single_packet=False)
                nc.gpsimd.dma_gather(
                    xg[:, K * nA:K * G, :],
                    xtB[:], ib[:], K * nB * 128, K * nB * 128, HC,
                    queue_num=(2 * sg + 1) % 4, single_packet=False)

                for bb in range(K):
                    b = sg * K + bb
                    part = xg[:].ap[0]
                    xg4A = vw(xg[:], [part, [HC, nA], [C, H], [1, C]],
                              extra_offset=bb * nA * HC)
                    xg4B = vw(xg[:], [part, [HC, nB], [C, H], [1, C]],
                              extra_offset=(K * nA + bb * nB) * HC)

                    rhs = bpool.tile([128, G, RW], BF16, tag="rhs")
                    nc.sync.dma_start(out=rhs[:, :, HC + H:RW], in_=eaones[b])
                    drr = spool.tile([1, G * 128], BF16, tag="drr")
                    nc.sync.dma_start(
                        out=drr[:],
                        in_=vw(dstrelR[sg], [[1, 1], [1, G * 128]],
                               extra_offset=bb * G * 128))
                    egt = spool.tile([ED, G, 128], BF16, tag="egt")
                    nc.sync.dma_start(
                        out=egt[:],
                        in_=vw(eaT[sg], [[K * G * 128, ED], [128, G], [1, 128]],
                               extra_offset=bb * G * 128))

                    # --- maskT [j, G, i] ---
                    maskT = bpool.tile([128, G, 128], BF16, tag="maskT")
                    drt_b = vw(drt[:], [drt[:].ap[0], [1, G], [0, 128]],
                               extra_offset=bb * G)
                    iota_b = vw(c_iota[:], [c_iota[:].ap[0], [0, G], [1, 128]])
                    nc.vector.tensor_tensor(out=maskT[:], in0=drt_b, in1=iota_b,
                                            op=mybir.AluOpType.is_equal)

                    # --- a_src_e [j, G, H] ---
                    t_as = bpool.tile([128, G, H, C], BF16, tag="t_as")
                    asrc_bA = vw(c_asrc[:], [c_asrc[:].ap[0], [0, nA], [C, H], [1, C]])
                    asrc_bB = vw(c_asrc[:], [c_asrc[:].ap[0], [0, nB], [C, H], [1, C]])
                    nc.vector.tensor_tensor(out=t_as[:, 0:nA], in0=xg4A,
                                            in1=asrc_bA, op=mybir.AluOpType.mult)
                    nc.vector.tensor_tensor(out=t_as[:, nA:G], in0=xg4B,
                                            in1=asrc_bB, op=mybir.AluOpType.mult)
                    a_src_e = mpool.tile([128, G, H], F32, tag="a_src_e")
                    nc.vector.reduce_sum(out=a_src_e[:], in_=t_as[:],
                                         axis=mybir.AxisListType.X)

                    # --- per-node a_dst/a_src (self rows) ---
                    t_bs = mpool.tile([128, 2, H, C], BF16, tag="t_bs")
                    xsb = vw(xs_sup[:], [xs_sup[:].ap[0], [0, 1], [C, H], [1, C]],
                             extra_offset=b * HC)
                    ad2 = vw(c_adst[:], [c_adst[:].ap[0], [0, 1], [C, H], [1, C]])
                    as2 = vw(c_asrc[:], [c_asrc[:].ap[0], [0, 1], [C, H], [1, C]])
                    nc.vector.tensor_tensor(out=t_bs[:, 0:1], in0=xsb, in1=ad2,
                                            op=mybir.AluOpType.mult)
                    nc.vector.tensor_tensor(out=t_bs[:, 1:2], in0=xsb, in1=as2,
                                            op=mybir.AluOpType.mult)
                    blkv = spool.tile([128, 2, H], F32, tag="blkv")
                    nc.vector.reduce_sum(out=blkv[:], in_=t_bs[:],
                                         axis=mybir.AxisListType.X)
                    a_dst_bf = spool.tile([128, H], BF16, tag="a_dst_bf")
                    nc.vector.tensor_copy(out=a_dst_bf[:], in_=blkv[:, 0, :])

                    # --- a_dst expansion: bcast matmul + is_equal + matmul ---
                    ps3 = psB.tile([128, G, H], F32, tag="ps3")
                    ps4 = psB.tile([128, G, H], F32, tag="ps4")
                    ngrp = (G + 3) // 4
                    for grp in range(ngrp):
                        g0 = grp * 4
                        gw = min(4, G - g0)
                        psb_t = psT.tile([128, 4, 128], F32, tag="psb")
                        drr_s = vw(drr[:], [drr[:].ap[0], [1, gw * 128]],
                                   extra_offset=g0 * 128)
                        nc.tensor.matmul(
                            out=vw(psb_t[:], [psb_t[:].ap[0], [1, gw * 128]]),
                            lhsT=c_ones[:], rhs=drr_s, start=True, stop=True)
                        psb_c = kpool.tile([128, 4, 128], BF16, tag="psb_c")
                        nc.scalar.copy(out=psb_c[:, 0:gw], in_=psb_t[:, 0:gw])
                        mask4 = kpool.tile([128, 4, 128], BF16, tag="mask4")
                        iotc_b = vw(c_iotc[:], [c_iotc[:].ap[0], [0, gw], [0, 128]])
                        nc.vector.tensor_tensor(
                            out=mask4[:, 0:gw], in0=psb_c[:, 0:gw], in1=iotc_b,
                            op=mybir.AluOpType.is_equal)
                        for gg in range(gw):
                            g = g0 + gg
                            nc.tensor.matmul(out=ps3[:, g, :],
                                             lhsT=mask4[:, gg, :],
                                             rhs=a_dst_bf[:],
                                             start=True, stop=True)
                            nc.tensor.matmul(out=ps4[:, g, :],
                                             lhsT=egt[:, g, :],
                                             rhs=c_v16[:],
                                             start=True, stop=True)

                    # --- alpha / ex ---
                    alpha = mpool.tile([128, G, H], F32, tag="alpha")
                    nc.vector.tensor_add(out=alpha[:], in0=ps3[:], in1=a_src_e[:])
                    nc.vector.tensor_add(out=alpha[:], in0=alpha[:], in1=ps4[:])
                    lrel = mpool.tile([128, G, H], F32, tag="lrel")
                    nc.vector.tensor_scalar_mul(out=lrel[:], in0=alpha[:],
                                                scalar1=NEG_ATT)
                    nc.vector.tensor_tensor(out=lrel[:], in0=lrel[:], in1=alpha[:],
                                            op=mybir.AluOpType.max)
                    nc.scalar.activation(out=rhs[:, :, HC:HC + H], in_=lrel[:],
                                         func=mybir.ActivationFunctionType.Exp)

                    # --- wmsg ---
                    ex_bA = vw(rhs[:], [rhs[:].ap[0], [RW, nA], [1, H], [0, C]],
                               extra_offset=HC)
                    ex_bB = vw(rhs[:], [rhs[:].ap[0], [RW, nB], [1, H], [0, C]],
                               extra_offset=nA * RW + HC)
                    woutA = vw(rhs[:], [rhs[:].ap[0], [RW, nA], [C, H], [1, C]])
                    woutB = vw(rhs[:], [rhs[:].ap[0], [RW, nB], [C, H], [1, C]],
                               extra_offset=nA * RW)
                    nc.vector.tensor_tensor(out=woutA, in0=xg4A, in1=ex_bA,
                                            op=mybir.AluOpType.mult)
                    nc.vector.tensor_tensor(out=woutB, in0=xg4B, in1=ex_bB,
                                            op=mybir.AluOpType.mult)

                    # --- main accumulation ---
                    psm = psA.tile([128, RW], F32, tag="psm")
                    for g in range(G):
                        nc.tensor.matmul(out=psm[:], lhsT=maskT[:, g, :],
                                         rhs=rhs[:, g, :],
                                         start=(g == 0), stop=(g == G - 1))

                    # --- self loop + normalize ---
                    cntc = spool.tile([128, 1], F32, tag="cntc")
                    nc.vector.tensor_scalar_max(out=cntc[:], in0=psm[:, RW - 1:RW],
                                                scalar1=1.0)
                    rcnt = spool.tile([128, 1], F32, tag="rcnt")
                    nc.vector.reciprocal(out=rcnt[:], in_=cntc[:])
                    lattr = spool.tile([128, ED], F32, tag="lattr")
                    nc.vector.tensor_scalar_mul(out=lattr[:],
                                                in0=psm[:, HC + H:HC + H + ED],
                                                scalar1=rcnt[:, 0:1])
                    t_al = spool.tile([128, H, ED], F32, tag="t_al")
                    lattr_b = vw(lattr[:], [lattr[:].ap[0], [0, H], [1, ED]])
                    v_b2 = vw(c_v[:], [c_v[:].ap[0], [ED, H], [1, ED]])
                    nc.vector.tensor_tensor(out=t_al[:], in0=lattr_b, in1=v_b2,
                                            op=mybir.AluOpType.mult)
                    a_el = spool.tile([128, H], F32, tag="a_el")
                    nc.vector.reduce_sum(out=a_el[:], in_=t_al[:],
                                         axis=mybir.AxisListType.X)
                    alf = spool.tile([128, H], F32, tag="alf")
                    nc.vector.tensor_add(out=alf[:], in0=blkv[:, 0, :],
                                         in1=blkv[:, 1, :])
                    nc.vector.tensor_add(out=alf[:], in0=alf[:], in1=a_el[:])
                    alf2 = spool.tile([128, H], F32, tag="alf2")
                    nc.vector.tensor_scalar_mul(out=alf2[:], in0=alf[:],
                                                scalar1=NEG_ATT)
                    nc.vector.tensor_tensor(out=alf2[:], in0=alf2[:], in1=alf[:],
                                            op=mybir.AluOpType.max)
                    exl = spool.tile([128, H], F32, tag="exl")
                    nc.scalar.activation(out=exl[:], in_=alf2[:],
                                         func=mybir.ActivationFunctionType.Exp)

                    den = spool.tile([128, H], F32, tag="den")
                    nc.vector.tensor_add(out=den[:], in0=psm[:, HC:HC + H],
                                         in1=exl[:])
                    rden = spool.tile([128, H], F32, tag="rden")
                    nc.vector.reciprocal(out=rden[:], in_=den[:])

                    smsg = mpool.tile([128, HC], F32, tag="smsg")
                    exl_b = vw(exl[:], [exl[:].ap[0], [1, H], [0, C]])
                    xs2 = vw(xs_sup[:], [xs_sup[:].ap[0], [C, H], [1, C]],
                             extra_offset=b * HC)
                    nc.vector.tensor_tensor(out=smsg[:], in0=xs2, in1=exl_b,
                                            op=mybir.AluOpType.mult)
                    agg = mpool.tile([128, HC], F32, tag="agg")
                    nc.vector.tensor_add(out=agg[:], in0=psm[:, 0:HC], in1=smsg[:])
                    rden_b = vw(rden[:], [rden[:].ap[0], [1, H], [0, C]])
                    nc.vector.tensor_tensor(out=agg[:], in0=agg[:], in1=rden_b,
                                            op=mybir.AluOpType.mult)
                    nc.vector.tensor_add(out=agg[:], in0=agg[:], in1=c_bias[:])
                    osb = mpool.tile([128, HC], F32, tag="osb")
                    nc.scalar.mul(out=osb[:], in_=agg[:], mul=NEG_OUT)
                    nc.vector.tensor_tensor(out=osb[:], in0=osb[:], in1=agg[:],
                                            op=mybir.AluOpType.max)
                    nc.sync.dma_start(out=out[b], in_=osb[:])

    nc.finalize()
    return nc


def assemble(cfg, meta, results):
    """Gather per-core outputs back to full [N, HC] float32."""
    NPAD = cfg.NPAD
    flat = np.zeros((NPAD, cfg.HC), np.float32)
    for c in range(CORES):
        o = results[c]["out"]  # [BPC, 128, HC]
        bl = meta["core_blocks"][c]
        for s, b in enumerate(bl):
            flat[b * 128:(b + 1) * 128] = o[s]
    y = np.empty((cfg.N, cfg.HC), np.float32)
    y[meta["inv"][meta["valid"]]] = flat[meta["valid"]]
    return y


_BUILD_CACHE = {}


def kernel(**inputs):
    cfg = full_cfg()
    in_maps, meta = prep(cfg, **inputs)
    ckey = (cfg.N, cfg.E, cfg.nA, cfg.nB)
    if ckey in _BUILD_CACHE:
        nc = _BUILD_CACHE[ckey]
    else:
        nc = build(cfg)
        _BUILD_CACHE[ckey] = nc
    res = run_bass_kernel_spmd(nc, in_maps, core_ids=list(range(CORES)))
    return assemble(cfg, meta, res.results)
